# revision 3
# baseline (speedup 1.0000x reference)
"""Trainium2 Bass kernel for a 2-layer GAT (4 heads, 32 dim/head) + linear classifier.

Architecture (8 NeuronCores, SPMD; 3 device launches with host-side edge
expansion between them — the host only permutes/replicates rows, all FLOPs
run on device):

- Host prep: append self-loops, sort edges by dst, partition dst nodes into
  8 x 49 blocks of 128 nodes; per-(core,block) edge lists padded to T_b x 128
  slots (T_b = max over cores, baked into the program as static loop bounds).
  Slot (block b, tile t, partition p) holds edge number t*128+p of block b.

- Launch A (node transform): each core computes rows[n] = x[n] @ [W1|W1@As1|W1@Ad1]
  for its 6272-node shard (x supplied pre-transposed so each 128-node tile is a
  single matmul with lhsT = xT tile). Output -> host.

- Host expansion 1: EA1[slot] = [h0[src] (128) | a_src1[src] (4) | a_dst1[dst] (4)]
  per core (pure indexing of launch-A output; this is the graph "halo exchange" /
  gather, done host-side because this stack has no usable HW gather: extended Q7
  gather instructions crash the device and indirect DMA costs ~15us/instruction).

- Launch B (GAT layer 1): per block: sequential DMA of the block's edge rows;
  w = exp(leakyrelu(a_src+a_dst)) computed batched; scale h rows by w (head-
  expanded broadcast AP); build one-hot A01[p,t,d] = (dstlocal==d) with a single
  DVE is_equal (broadcast APs); per-tile PE matmul lhsT=A01, rhs=[h*w|w] accumulates
  [128,132] in PSUM = weighted feature sums + softmax denominators; node phase
  h1 = relu(U/(s+1e-16) + b1); PE-transpose h1; z2rows = h1 @ [W2|W2@As2|W2@Ad2].

- Host expansion 2, then Launch C: same aggregation for layer 2, node phase
  h2 = relu(U2/s2 + b2), logits = h2 @ Wc + bc -> out [6272, 40] per core.
"""

import os
import sys
import time

for _p in ("/opt/trn_rl_repo", "/root/.axon_site/_ro/trn_rl_repo"):
    if os.path.isdir(_p) and _p not in sys.path:
        sys.path.insert(0, _p)

import dataclasses

import numpy as np

import concourse.bass as bass
import concourse.mybir as mybir
import concourse.tile as tile
from concourse import bacc
from concourse.bass_utils import run_bass_kernel_spmd

P = 128
D = 128
HEADS = 4
C = 40
NEG_SLOPE = 0.2
ROW = D + 2 * HEADS  # 136: [h | a_src | a_dst]
NCORES = 8

f32 = mybir.dt.float32
i32 = mybir.dt.int32

LAST_INFO = {}  # timing info stash for test.py

# Tunable implementation choices (A/B-tested on HW):
#   gw_eng: engine for the big h*w multiply: "gp" | "dve" | "split"
#   lrelu_act: leaky-relu on ACT (Lrelu) instead of 2 DVE ops
#   mm: aggregation-matmul dtype: "f32" | "f32r" | "bf16"
CONFIG = {"gw_eng": "dve", "lrelu_act": False, "mm": "f32", "a01_eng": "dve",
          "smalls": "dve"}

bf16 = mybir.dt.bfloat16


def _ap_with(ap, dims):
    return dataclasses.replace(ap, ap=dims)


def _head_expand(w_ap, T):
    """[P, T*4] AP -> [P, T, 4, 32] broadcast AP (each head value repeated 32x)."""
    s = w_ap.ap[1][0]
    return _ap_with(w_ap, [w_ap.ap[0], [s * 4, T], [s, 4], [0, 32]])


def build_node_transform(nper, wcols, repeat=1, trn_type="TRN2"):
    """Launch A: rows[n] = xTs[:, n].T @ wcat  for n in [0, nper)."""
    nc = bacc.Bacc(trn_type, target_bir_lowering=False, debug=False, num_devices=NCORES)
    xts_d = nc.dram_tensor("xts", [P, nper], f32, kind="ExternalInput")
    wcat_d = nc.dram_tensor("wcat", [D, wcols], f32, kind="ExternalInput")
    out_d = nc.dram_tensor("nrows", [nper, wcols], f32, kind="ExternalOutput")
    nt = nper // P
    with tile.TileContext(nc) as tc:
        with (
            tc.tile_pool(name="const", bufs=1) as cpool,
            tc.tile_pool(name="work", bufs=4) as pool,
            tc.tile_pool(name="psum", bufs=4, space="PSUM") as psum,
        ):
            wcat = cpool.tile([D, wcols], f32, tag="wcat")
            nc.sync.dma_start(wcat[:], wcat_d[:])
            for rep in range(repeat):
                if rep:
                    tc.strict_bb_all_engine_barrier()
                for i in range(nt):
                    xt = pool.tile([P, P], f32, tag="xt")
                    nc.sync.dma_start(xt[:], xts_d[:, i * P : (i + 1) * P])
                    hp = psum.tile([P, wcols], f32, tag="hp")
                    nc.tensor.matmul(hp[:], lhsT=xt[:], rhs=wcat[:], start=True, stop=True)
                    hf = pool.tile([P, wcols], f32, tag="hf")
                    nc.vector.tensor_copy(hf[:], hp[:])
                    nc.sync.dma_start(out_d[i * P : (i + 1) * P, :], hf[:])
    nc.compile()
    return nc


def build_gat_layer(nper, Ts, wcols, is_last, repeat=1, trn_type="TRN2", cfg=None):
    """Launch B/C: edge aggregation + node phase for one GAT layer.

    Inputs: EA [TOT*128, 136] edge rows (slot-major), meta [P, TOT] (dstlocal
    f32 bits as int32), wnext [D, wcols], brep [P, D] bias, ident [P, P].
    Output: zrows [nper, wcols] (B) or logits out [nper, C] (C, wcols=C).
    """
    cfg = dict(CONFIG if cfg is None else cfg)
    abl = set(cfg.get("ablate", ()))
    edt = bf16 if cfg["mm"] == "bf16" else f32
    nblocks = nper // P
    assert len(Ts) == nblocks
    TOT = int(sum(Ts))
    offs = np.concatenate([[0], np.cumsum(Ts)]).astype(int)

    nc = bacc.Bacc(trn_type, target_bir_lowering=False, debug=False, num_devices=NCORES)
    ea_d = nc.dram_tensor("ea", [TOT * P, ROW], edt, kind="ExternalInput")
    meta_d = nc.dram_tensor("meta", [P, TOT], edt, kind="ExternalInput")
    wnext_d = nc.dram_tensor("wnext", [D, wcols], f32, kind="ExternalInput")
    brep_d = nc.dram_tensor("brep", [P, D], f32, kind="ExternalInput")
    bout_d = nc.dram_tensor("bout", [P, wcols], f32, kind="ExternalInput")
    iota_d = nc.dram_tensor("iota", [P, P], edt, kind="ExternalInput")
    ident_d = nc.dram_tensor("ident", [P, P], f32, kind="ExternalInput")
    out_d = nc.dram_tensor("zrows", [nper, wcols], f32, kind="ExternalOutput")

    ea3 = ea_d[:].rearrange("(t p) c -> p t c", p=P)  # slot (p,t) = EA[t*128+p]

    with tile.TileContext(nc) as tc:
        with (
            tc.tile_pool(name="const", bufs=1) as cpool,
            tc.tile_pool(name="work", bufs=int(cfg.get("wbufs", 3))) as pool,
            tc.tile_pool(name="gath", bufs=int(cfg.get("gbufs", 3))) as gpool,
            tc.tile_pool(name="psum", bufs=2, space="PSUM") as psum,
            tc.tile_pool(name="psU", bufs=int(cfg.get("ubufs", 2)), space="PSUM") as psU,
        ):
            wnext = cpool.tile([D, wcols], f32, tag="wnext")
            nc.sync.dma_start(wnext[:], wnext_d[:])
            brep = cpool.tile([P, D], f32, tag="brep")
            nc.sync.dma_start(brep[:], brep_d[:])
            bout = cpool.tile([P, wcols], f32, tag="bout")
            nc.sync.dma_start(bout[:], bout_d[:])
            iota = cpool.tile([P, P], edt, tag="iota")
            nc.sync.dma_start(iota[:], iota_d[:])
            ident = cpool.tile([P, P], f32, tag="ident")
            nc.sync.dma_start(ident[:], ident_d[:])

            for rep in range(repeat):
                if rep:
                    tc.strict_bb_all_engine_barrier()
                for b in range(nblocks):
                    T = int(Ts[b])
                    off = int(offs[b])
                    mt = pool.tile([P, T], edt, tag="meta")
                    nc.sync.dma_start(mt[:], meta_d[:, off : off + T])

                    G = gpool.tile([P, T * ROW], edt, tag="G")
                    g3 = G[:].rearrange("p (t c) -> p t c", c=ROW)
                    if "dma" not in abl:
                        nc.sync.dma_start(g3, ea3[:, off : off + T, :])
                    elif b == 0 and rep == 0:
                        nc.sync.dma_start(g3, ea3[:, 0:T, :])

                    acols = g3[:, :, D : D + HEADS]  # a_src
                    dcols = g3[:, :, D + HEADS : ROW]  # a_dst (of dst node)
                    hcols = g3[:, :, 0:D]

                    # e = a_src + a_dst ; leaky ; w = exp(e)
                    ew = pool.tile([P, T * HEADS], f32, tag="ew")
                    sm_eng = nc.gpsimd if cfg.get("smalls") == "spread" else nc.vector
                    if "smalls" in abl:
                        nc.vector.memset(ew[:], 0.5)
                    else:
                        sm_eng.tensor_tensor(out=ew[:], in0=acols, in1=dcols, op=mybir.AluOpType.add)
                    if "smalls" in abl:
                        pass
                    elif cfg["lrelu_act"]:
                        nc.scalar.activation(ew[:], ew[:], mybir.ActivationFunctionType.Lrelu,
                                             alpha=NEG_SLOPE)
                    else:
                        lk = pool.tile([P, T * HEADS], f32, tag="lk")
                        sm_eng.tensor_scalar(lk[:], ew[:], NEG_SLOPE, None, mybir.AluOpType.mult)
                        sm_eng.tensor_tensor(out=ew[:], in0=ew[:], in1=lk[:], op=mybir.AluOpType.max)
                    w = pool.tile([P, T * HEADS], edt, tag="w")
                    nc.scalar.activation(w[:], ew[:], mybir.ActivationFunctionType.Exp)

                    # rhs = [h*w | w]
                    mdt = bf16 if cfg["mm"] in ("bf16", "bf16mm") else f32
                    if cfg.get("gw_sep", True):
                        RW = D + HEADS
                        GW = gpool.tile([P, T * RW], mdt, tag="GW")
                        gw3 = GW[:].rearrange("p (t c) -> p t c", c=RW)
                        rhs_src = gw3
                        if "gw" not in abl:
                            nc.vector.tensor_tensor(
                                out=gw3[:, :, 0:D], in0=hcols, in1=_head_expand(w[:], T),
                                op=mybir.AluOpType.mult,
                            )
                        if "smalls" not in abl:
                            nc.vector.tensor_copy(out=gw3[:, :, D:RW], in_=w[:])
                    else:
                        rhs_src = g3
                        if "gw" not in abl:
                            nc.vector.tensor_tensor(
                                out=hcols, in0=hcols, in1=_head_expand(w[:], T), op=mybir.AluOpType.mult
                            )
                        if "smalls" not in abl:
                            nc.vector.tensor_copy(out=acols, in_=w[:])

                    # one-hot A01[p, t, d] = (dstloc[p,t] == d)
                    dstloc = mt[:]
                    A01 = gpool.tile([P, T * P], mdt, tag="A01")
                    a01_eng = cfg.get("a01_eng", "dve")
                    if "a01" in abl:
                        nc.vector.memset(A01[:, 0:P], 0.0)
                    elif a01_eng == "split":
                        th = T // 2
                        for eng, sl in ((nc.vector, slice(0, th)), (nc.gpsimd, slice(th, T))):
                            tn = sl.stop - sl.start
                            if tn <= 0:
                                continue
                            dl = dstloc[:, sl]
                            i0 = _ap_with(dl, [dl.ap[0], [dl.ap[1][0], tn], [0, P]])
                            i1 = _ap_with(iota[:], [iota[:].ap[0], [0, tn], [iota[:].ap[1][0], P]])
                            o = A01[:, sl.start * P : sl.stop * P]
                            eng.tensor_tensor(out=o, in0=i0, in1=i1, op=mybir.AluOpType.is_equal)
                    else:
                        eng = nc.vector if a01_eng == "dve" else nc.gpsimd
                        in0 = _ap_with(dstloc, [dstloc.ap[0], [dstloc.ap[1][0], T], [0, P]])
                        in1 = _ap_with(iota[:], [iota[:].ap[0], [0, T], [iota[:].ap[1][0], P]])
                        eng.tensor_tensor(out=A01[:], in0=in0, in1=in1, op=mybir.AluOpType.is_equal)

                    # aggregate: U = sum_t A01_t.T @ [h*w | w]_t
                    a3 = A01[:].rearrange("p (t d) -> p t d", d=P)
                    U = psU.tile([P, D + HEADS], f32, tag="U")
                    f32r = mybir.dt.float32r
                    for t in range(1 if "mm" in abl else T):
                        lhsT_t = a3[:, t, :]
                        rhs_t = rhs_src[:, t, 0 : D + HEADS]
                        if cfg["mm"] == "f32r":
                            lhsT_t = lhsT_t.bitcast(f32r)
                            rhs_t = rhs_t.bitcast(f32r)
                        nc.tensor.matmul(
                            U[:], lhsT=lhsT_t, rhs=rhs_t,
                            start=(t == 0), stop=(t == T - 1) or "mm" in abl,
                        )

                    # node phase: h = relu(U/(s+eps) + b)
                    s_eps = pool.tile([P, HEADS], f32, tag="s_eps")
                    nc.vector.tensor_scalar_add(s_eps[:], U[:, D : D + HEADS], 1e-16)
                    rcp = pool.tile([P, HEADS], f32, tag="rcp")
                    nc.vector.reciprocal(rcp[:], s_eps[:])
                    h = pool.tile([P, D], f32, tag="h")
                    nc.vector.tensor_tensor(
                        out=h[:], in0=U[:, 0:D], in1=_head_expand(rcp[:], 1), op=mybir.AluOpType.mult
                    )
                    nc.vector.tensor_tensor(out=h[:], in0=h[:], in1=brep[:], op=mybir.AluOpType.add)
                    nc.scalar.activation(h[:], h[:], mybir.ActivationFunctionType.Relu)

                    # project: zrows = h @ wnext (+0; bias bc handled via brep-less add for C)
                    hTp = psum.tile([P, P], f32, tag="hTp")
                    nc.tensor.transpose(hTp[:], h[:], ident[:])
                    hT = pool.tile([P, P], f32, tag="hT")
                    nc.vector.tensor_copy(hT[:], hTp[:])
                    zp = psum.tile([P, wcols], f32, tag="zp")
                    nc.tensor.matmul(zp[:], lhsT=hT[:], rhs=wnext[:], start=True, stop=True)
                    z = pool.tile([P, wcols], f32, tag="z")
                    nc.vector.tensor_tensor(out=z[:], in0=zp[:], in1=bout[:], op=mybir.AluOpType.add)
                    nc.sync.dma_start(out_d[b * P : (b + 1) * P, :], z[:])

    nc.compile()
    return nc


def prep_edges(edge_index, n, ncores):
    """Sort self-looped edges by dst; per-core slot layout. Returns
    (Ts, src_slots[ncores], dst_slots[ncores], metas[ncores], npad)."""
    nper_raw = -(-n // (ncores * P)) * P
    npad = nper_raw * ncores
    nper = nper_raw
    nblocks = nper // P

    e0 = np.asarray(edge_index[0], dtype=np.int64)
    e1 = np.asarray(edge_index[1], dtype=np.int64)
    loops = np.arange(n, dtype=np.int64)
    src = np.concatenate([e0, loops])
    dst = np.concatenate([e1, loops])
    order = np.argsort(dst, kind="stable")
    srcs = src[order].astype(np.int64)
    dsts = dst[order].astype(np.int64)

    bounds = np.searchsorted(dsts, np.arange(0, npad + 1, P))
    counts = (bounds[1:] - bounds[:-1]).reshape(ncores, nblocks)
    Ts = np.maximum(1, -(-counts.max(axis=0) // P))  # [nblocks]
    TOT = int(Ts.sum())
    offs = np.concatenate([[0], np.cumsum(Ts)]).astype(int)

    src_slots, dst_slots, metas = [], [], []
    for c in range(ncores):
        ss = np.zeros(TOT * P, dtype=np.int64)
        ds_ = np.zeros(TOT * P, dtype=np.int64)
        meta = np.full((P, TOT), 300.0, dtype=np.float32)
        for b in range(nblocks):
            g = c * nblocks + b
            lo, hi = int(bounds[g]), int(bounds[g + 1])
            cnt = hi - lo
            T = int(Ts[b])
            base = int(offs[b]) * P
            # edge j of block -> slot base + t*128 + p with p=j%128, t=j//128
            j = np.arange(cnt)
            slot = base + (j // P) * P + (j % P)
            ss[slot] = srcs[lo:hi]
            ds_[slot] = dsts[lo:hi]
            loc = (dsts[lo:hi] - g * P).astype(np.float32)
            meta[j % P, int(offs[b]) + j // P] = loc
        src_slots.append(ss)
        dst_slots.append(ds_)
        metas.append(np.ascontiguousarray(meta))
    return Ts, src_slots, dst_slots, metas, npad


def expand_rows(nrows_full, src_slots, dst_slots):
    """EA[slot] = [nrows[src][0:132] | nrows[dst][132:136]] per core."""
    eas = []
    for ss, ds_ in zip(src_slots, dst_slots):
        ea = nrows_full[ss].copy()
        ea[:, D + HEADS : ROW] = nrows_full[ds_, D + HEADS : ROW]
        eas.append(ea)
    return eas


def amat(att):
    A = np.zeros((D, HEADS), dtype=np.float32)
    att = np.asarray(att, dtype=np.float32)
    for h in range(HEADS):
        A[h * (D // HEADS) : (h + 1) * (D // HEADS), h] = att[h]
    return A


_cache = {}


def run_gat(x, edge_index, W1, att_src1, att_dst1, b1, W2, att_src2, att_dst2, b2,
            Wc, bc, n=None, ncores=NCORES, repeat=1):
    global LAST_INFO
    x = np.asarray(x, dtype=np.float32)
    if n is None:
        n = int(x.shape[0])

    t0 = time.time()
    Ts, src_slots, dst_slots, metas, npad = prep_edges(edge_index, n, ncores)
    nper = npad // ncores
    cfg = dict(CONFIG)
    key = (npad, tuple(Ts), ncores, repeat, tuple(sorted(cfg.items())))
    t1 = time.time()
    if key in _cache:
        ncA, ncB, ncC = _cache[key]
    else:
        ncA = build_node_transform(nper, ROW)
        ncB = build_gat_layer(nper, Ts, ROW, is_last=False, repeat=repeat, cfg=cfg)
        ncC = build_gat_layer(nper, Ts, C, is_last=True, repeat=repeat, cfg=cfg)
        _cache[key] = (ncA, ncB, ncC)
    t2 = time.time()

    W1 = np.asarray(W1, dtype=np.float32)
    W2 = np.asarray(W2, dtype=np.float32)
    Wc = np.asarray(Wc, dtype=np.float32)
    w1cat = np.ascontiguousarray(np.concatenate([W1, W1 @ amat(att_src1), W1 @ amat(att_dst1)], axis=1))
    w2cat = np.ascontiguousarray(np.concatenate([W2, W2 @ amat(att_src2), W2 @ amat(att_dst2)], axis=1))
    b1r = np.tile(np.asarray(b1, np.float32)[None, :], (P, 1))
    b2r = np.tile(np.asarray(b2, np.float32)[None, :], (P, 1))
    bc = np.asarray(bc, dtype=np.float32)
    iota = np.tile(np.arange(P, dtype=np.float32), (P, 1))
    ident = np.eye(P, dtype=np.float32)
    if cfg["mm"] == "bf16":
        import ml_dtypes
        iota = iota.astype(ml_dtypes.bfloat16)
        metas = [m.astype(ml_dtypes.bfloat16) for m in metas]

    xp = np.zeros((npad, D), dtype=np.float32)
    xp[:n] = x

    # Launch A
    mapsA = [
        {"xts": np.ascontiguousarray(xp[c * nper : (c + 1) * nper].T), "wcat": w1cat}
        for c in range(ncores)
    ]
    resA = run_bass_kernel_spmd(ncA, mapsA, list(range(ncores)))
    nrows_full = np.concatenate([resA.results[c]["nrows"] for c in range(ncores)], axis=0)
    t3 = time.time()

    # Expansion 1 + Launch B
    eas = expand_rows(nrows_full, src_slots, dst_slots)
    if cfg["mm"] == "bf16":
        import ml_dtypes
        eas = [e.astype(ml_dtypes.bfloat16) for e in eas]
    mapsB = [
        {"ea": eas[c], "meta": metas[c], "wnext": w2cat, "brep": b1r,
         "bout": np.zeros((P, ROW), np.float32), "iota": iota, "ident": ident}
        for c in range(ncores)
    ]
    resB = run_bass_kernel_spmd(ncB, mapsB, list(range(ncores)))
    zrows_full = np.concatenate([resB.results[c]["zrows"] for c in range(ncores)], axis=0)
    t4 = time.time()

    # Expansion 2 + Launch C
    eas2 = expand_rows(zrows_full, src_slots, dst_slots)
    if cfg["mm"] == "bf16":
        import ml_dtypes
        eas2 = [e.astype(ml_dtypes.bfloat16) for e in eas2]
    mapsC = [
        {"ea": eas2[c], "meta": metas[c], "wnext": Wc, "brep": b2r,
         "bout": np.tile(bc[None, :], (P, 1)), "iota": iota, "ident": ident}
        for c in range(ncores)
    ]
    resC = run_bass_kernel_spmd(ncC, mapsC, list(range(ncores)))
    out = np.concatenate([resC.results[c]["zrows"] for c in range(ncores)], axis=0)[:n]
    t5 = time.time()

    LAST_INFO = {
        "prep_s": t1 - t0, "build_s": t2 - t1, "launchA_s": t3 - t2,
        "launchB_s": t4 - t3, "launchC_s": t5 - t4,
        "ncs": (ncA, ncB, ncC),
        "maps": (mapsA, mapsB, mapsC),
        "nper": nper, "Ts": Ts,
    }
    print(
        f"[kernel] prep={t1 - t0:.2f}s build={t2 - t1:.2f}s A={t3 - t2:.2f}s "
        f"B={t4 - t3:.2f}s C={t5 - t4:.2f}s",
        file=sys.stderr,
    )
    return out.astype(np.float32)


def kernel(x, edge_index, W1, att_src1, att_dst1, b1, W2, att_src2, att_dst2, b2, Wc, bc):
    return run_gat(x, edge_index, W1, att_src1, att_dst1, b1,
                   W2, att_src2, att_dst2, b2, Wc, bc)



# revision 25
# speedup vs baseline: 955.8085x; 955.8085x over previous
"""Trainium2 Bass kernel for a 2-layer GAT (4 heads, 32 dim/head) + linear classifier.

Architecture (8 NeuronCores, SPMD; 3 device launches with host-side edge
expansion between them — the host only permutes/replicates rows, all FLOPs
run on device):

- Host prep: append self-loops, sort edges by dst, partition the 392 dst
  node-blocks (128 nodes each) across 8 cores x 49 positions, balancing edge
  counts (blocks sorted by count, dealt round-robin) to minimize the shared
  per-position tile count Ts[b] = ceil(max-core count/128).

- Launch A (node transform): each core computes rows[n] = x[n] @ [W1|W1@As1|W1@Ad1]
  for its 6272-node shard, all bf16 (output rows [h | a_src | a_dst], 136 cols).

- Host expansion (per layer): gather per-edge rows EA[slot] =
  [h[src] | a_src[src] | a_dst[dst]] in a FEATURE-MAJOR layout per block:
  ea[p, (off_b + c)*T_b + t] = payload c of edge slot (b, t, p), bf16.
  Feature-major keeps every device elementwise op's innermost stride == 1,
  which is what unlocks the DVE 2x perf mode (broadcast APs force 1x).

- Launch B/C (GAT layer): per block:
    DMA G [P, 136*T];  ew = a_src+a_dst (DVE 2x);  lrelu+exp on ACT into the
    a_src region (w, head-major);  G.h *= w (DVE 2x, broadcast-free via
    4-dim AP);  one-hot A01[p, d*T+t] = (dstloc==d) via is_equal with a
    materialized wide-iota constant (packed operands -> 2x), split between
    DVE and GPSIMD;  T PE matmuls accumulate U[128,132] = A01^T @ [h*w | w];
    node phase h = relu(U/(s+eps) + b) (DVE + ACT);  PE transpose;
    projection matmul to the next layer's rows (B: bf16 out, C: logits+bc).
  Outputs are partition-major [P, nblocks*wcols] so per-block stores DMA
  contiguous per-partition chunks; the host unpacks.
"""

import os
import sys
import time

for _p in ("/opt/trn_rl_repo", "/root/.axon_site/_ro/trn_rl_repo"):
    if os.path.isdir(_p) and _p not in sys.path:
        sys.path.insert(0, _p)

import dataclasses

import numpy as np
import ml_dtypes

import concourse.bass as bass
import concourse.mybir as mybir
import concourse.tile as tile
from concourse import bacc
from concourse.bass_utils import run_bass_kernel_spmd

P = 128
D = 128
HEADS = 4
C = 40
NEG_SLOPE = 0.2
ROW = D + 2 * HEADS  # 136: [h | a_src | a_dst]
RHS = D + HEADS      # 132: [h*w | w] aggregation rhs width
NCORES = 8

f32 = mybir.dt.float32
bf16 = mybir.dt.bfloat16
bfnp = ml_dtypes.bfloat16

LAST_INFO = {}  # timing info stash for test.py

# gwp: number of h*w feature columns (multiple of 32) multiplied on GPSIMD;
# the rest go on DVE. The one-hot is_equal is DVE-only (GPSIMD lacks the op).
CONFIG = {"gwp": 0, "wbufs": 6, "gbufs": 5, "ubufs": 4}
for _k in ("gwp", "wbufs", "gbufs", "ubufs"):
    if os.environ.get(f"GAT_{_k.upper()}"):
        CONFIG[_k] = int(os.environ[f"GAT_{_k.upper()}"])


def _ap_with(ap, dims):
    return dataclasses.replace(ap, ap=dims)


def build_node_transform(nper, wcols, repeat=1, trn_type="TRN2"):
    """Launch A: rows[n] = xTs[:, n].T @ wcat, partition-major output
    npm[p, i*wcols + c] = rows[i*128 + p, c]; single wide in/out DMAs."""
    nc = bacc.Bacc(trn_type, target_bir_lowering=False, debug=False, num_devices=NCORES)
    xts_d = nc.dram_tensor("xts", [P, nper], bf16, kind="ExternalInput")
    wcat_d = nc.dram_tensor("wcat", [D, wcols], bf16, kind="ExternalInput")
    out_d = nc.dram_tensor("npm", [P, (nper // P) * wcols], bf16,
                           kind="ExternalOutput")
    nt = nper // P
    with tile.TileContext(nc) as tc:
        with (
            tc.tile_pool(name="const", bufs=1) as cpool,
            tc.tile_pool(name="work", bufs=2) as pool,
            tc.tile_pool(name="psum", bufs=4, space="PSUM") as psum,
        ):
            wcat = cpool.tile([D, wcols], bf16, tag="wcat")
            nc.sync.dma_start(wcat[:], wcat_d[:])
            for rep in range(repeat):
                if rep:
                    tc.strict_bb_all_engine_barrier()
                xts = pool.tile([P, nper], bf16, tag="xts")
                nc.sync.dma_start(xts[:], xts_d[:])
                ob = pool.tile([P, nt * wcols], bf16, tag="ob")
                for i in range(nt):
                    hp = psum.tile([P, wcols], f32, tag="hp")
                    nc.tensor.matmul(hp[:], lhsT=xts[:, i * P : (i + 1) * P],
                                     rhs=wcat[:], start=True, stop=True)
                    nc.vector.tensor_copy(ob[:, i * wcols : (i + 1) * wcols], hp[:])
                nc.sync.dma_start(out_d[:], ob[:])
    nc.compile()
    return nc


def build_gat_layer(nper, Ts, wcols, is_last, repeat=1, trn_type="TRN2", cfg=None):
    """Launch B/C: edge aggregation + node phase for one GAT layer (v2).

    Inputs (all per core): ea [P, TOT*136] bf16 feature-major edge rows,
    meta [P, TOT] bf16 dst-local ids (pad 300), iotat [P, 128*Tmax] bf16,
    wnext [128, wcols] bf16, brep [P, 128] bf16, bout [P, wcols] f32,
    ident [P, 128] bf16.
    Output: zpm [P, nblocks*wcols] partition-major (bf16 for B, f32 for C).
    """
    cfg = dict(CONFIG if cfg is None else cfg)
    gwp = int(cfg["gwp"])
    assert gwp % (D // HEADS) == 0
    abl = set(cfg.get("ablate", ()))  # timing-only ablations, break numerics
    ZCH = 8  # blocks per output-DMA chunk
    nblocks = nper // P
    assert len(Ts) == nblocks
    Ts = [int(t) for t in Ts]
    TOT = int(sum(Ts))
    Tmax = int(max(Ts))
    offs = np.concatenate([[0], np.cumsum(Ts)]).astype(int)
    odt = f32 if is_last else bf16

    nc = bacc.Bacc(trn_type, target_bir_lowering=False, debug=False, num_devices=NCORES)
    ea_d = nc.dram_tensor("ea", [P, TOT * ROW], bf16, kind="ExternalInput")
    meta_d = nc.dram_tensor("meta", [P, TOT], bf16, kind="ExternalInput")
    iotat_d = nc.dram_tensor("iotat", [P, P * Tmax], bf16, kind="ExternalInput")
    wnext_d = nc.dram_tensor("wnext", [D, wcols], bf16, kind="ExternalInput")
    brep_d = nc.dram_tensor("brep", [P, D], bf16, kind="ExternalInput")
    bout_d = nc.dram_tensor("bout", [P, wcols], f32, kind="ExternalInput")
    ident_d = nc.dram_tensor("ident", [P, P], bf16, kind="ExternalInput")
    out_d = nc.dram_tensor("zpm", [P, nblocks * wcols], odt, kind="ExternalOutput")

    AF = mybir.ActivationFunctionType

    with tile.TileContext(nc) as tc:
        with (
            tc.tile_pool(name="const", bufs=1) as cpool,
            tc.tile_pool(name="work", bufs=int(cfg["wbufs"])) as pool,
            tc.tile_pool(name="gath", bufs=int(cfg["gbufs"])) as gpool,
            tc.tile_pool(name="psum", bufs=2, space="PSUM") as psum,
            tc.tile_pool(name="psU", bufs=int(cfg["ubufs"]), space="PSUM") as psU,
        ):
            wnext = cpool.tile([D, wcols], bf16, tag="wnext")
            nc.sync.dma_start(wnext[:], wnext_d[:])
            brep = cpool.tile([P, D], bf16, tag="brep")
            nc.sync.dma_start(brep[:], brep_d[:])
            bout = cpool.tile([P, wcols], f32, tag="bout")
            nc.sync.dma_start(bout[:], bout_d[:])
            ident = cpool.tile([P, P], bf16, tag="ident")
            nc.sync.dma_start(ident[:], ident_d[:])
            iotat = cpool.tile([P, P * Tmax], bf16, tag="iotat")
            nc.sync.dma_start(iotat[:], iotat_d[:])
            meta = cpool.tile([P, TOT], bf16, tag="meta")
            nc.sync.dma_start(meta[:], meta_d[:])

            sp_iota = iotat[:].ap[0]
            H32 = D // HEADS

            for rep in range(repeat):
                if rep:
                    tc.strict_bb_all_engine_barrier()
                sts = {}
                zstate = {}

                def emit_dma(b):
                    T = Ts[b]
                    off = int(offs[b])
                    G = gpool.tile([P, ROW * T], bf16, tag="G")
                    if "dma" not in abl or (b == 0 and rep == 0):
                        nc.sync.dma_start(G[:], ea_d[:, off * ROW : (off + T) * ROW])
                    sts[b] = {"G": G, "T": T, "off": off}

                def emit_edge(b):
                    st = sts[b]
                    G, T, off = st["G"], st["T"], st["off"]
                    acol = G[:, D * T : (D + HEADS) * T]       # a_src, [P, 4T]
                    dcol = G[:, (D + HEADS) * T : ROW * T]     # a_dst, [P, 4T]
                    # e = a_src + a_dst; w = exp(leakyrelu(e)) -> over a_src
                    if "act" not in abl:
                        ew = pool.tile([P, HEADS * T], bf16, tag="ew")
                        nc.vector.tensor_tensor(out=ew[:], in0=acol, in1=dcol,
                                                op=mybir.AluOpType.add)
                        # leaky-relu via mult+max (ACT Lrelu ignores alpha)
                        lk = pool.tile([P, HEADS * T], bf16, tag="lk")
                        nc.vector.tensor_scalar_mul(lk[:], ew[:], NEG_SLOPE)
                        nc.vector.tensor_tensor(out=ew[:], in0=ew[:], in1=lk[:],
                                                op=mybir.AluOpType.max)
                        nc.scalar.activation(acol, ew[:], AF.Exp)
                    # h *= w (head-broadcast AP; packed last dims -> DVE 2x)
                    if "gw" not in abl:
                        for eng, c0, c1 in ((nc.vector, 0, D - gwp),
                                            (nc.gpsimd, D - gwp, D)):
                            if c1 <= c0:
                                continue
                            hpart = G[:, c0 * T : c1 * T]
                            wpart = _ap_with(G[:, (D + c0 // H32) * T :],
                                             [G[:].ap[0], [T, (c1 - c0) // H32],
                                              [0, H32], [1, T]])
                            eng.tensor_tensor(out=hpart, in0=hpart, in1=wpart,
                                              op=mybir.AluOpType.mult)
                    # one-hot A01[p, d*T + t] = (dstloc[p,t] == d)  (DVE 2x)
                    A01 = gpool.tile([P, P * T], bf16, tag="A01")
                    if "a01" in abl:
                        nc.vector.memset(A01[:], 0.0)
                    else:
                        m_b = meta[:, off : off + T]
                        i0 = _ap_with(m_b, [m_b.ap[0], [0, P], [1, T]])
                        i1 = _ap_with(iotat[:], [sp_iota, [Tmax, P], [1, T]])
                        o = _ap_with(A01[:], [A01[:].ap[0], [T, P], [1, T]])
                        nc.vector.tensor_tensor(out=o, in0=i0, in1=i1,
                                                op=mybir.AluOpType.is_equal)
                    st["A01"] = A01

                def emit_agg(b):
                    st = sts[b]
                    G, A01, T = st["G"], st["A01"], st["T"]
                    U = psU.tile([P, RHS], f32, tag="U")
                    for t in range(1 if "mm" in abl else T):
                        lhsT_t = _ap_with(A01[:, t:], [A01[:].ap[0], [T, P]])
                        rhs_t = _ap_with(G[:, t:], [G[:].ap[0], [T, RHS]])
                        nc.tensor.matmul(U[:], lhsT=lhsT_t, rhs=rhs_t,
                                         start=(t == 0),
                                         stop=(t == T - 1) or "mm" in abl)
                    st["U"] = U

                def emit_node(b):
                    st = sts.pop(b)
                    U = st["U"]
                    # node phase: h = relu(U/s + bias); s>0 for any node with
                    # an edge (self-loops guarantee it); pad rows give nan but
                    # are never gathered downstream.
                    hf = pool.tile([P, D], bf16, tag="hf")
                    if "node" in abl:
                        nc.vector.memset(hf[:], 0.0)
                    else:
                        rcp = pool.tile([P, HEADS], f32, tag="rcp")
                        nc.vector.reciprocal(rcp[:], U[:, D : D + HEADS])
                        h = pool.tile([P, D], bf16, tag="h")
                        rexp = _ap_with(rcp[:],
                                        [rcp[:].ap[0], [1, HEADS], [0, H32]])
                        nc.vector.tensor_tensor(out=h[:], in0=U[:, 0:D], in1=rexp,
                                                op=mybir.AluOpType.mult)
                        nc.vector.tensor_tensor(out=h[:], in0=h[:], in1=brep[:],
                                                op=mybir.AluOpType.add)
                        nc.scalar.activation(hf[:], h[:], AF.Relu)
                    # project: z = hf @ wnext (+ bout for the classifier)
                    hTp = psum.tile([P, P], bf16, tag="hTp")
                    nc.tensor.transpose(hTp[:], hf[:], ident[:])
                    hT = pool.tile([P, P], bf16, tag="hT")
                    nc.scalar.activation(hT[:], hTp[:], AF.Copy)
                    zp = psum.tile([P, wcols], f32, tag="zp")
                    nc.tensor.matmul(zp[:], lhsT=hT[:], rhs=wnext[:], start=True,
                                     stop=True)
                    if b % ZCH == 0:
                        zstate["b0"] = b
                        zstate["buf"] = pool.tile([P, ZCH * wcols], odt, tag="zbuf")
                    zb0, zbuf = zstate["b0"], zstate["buf"]
                    zs = zbuf[:, (b - zb0) * wcols : (b - zb0 + 1) * wcols]
                    if is_last:
                        nc.vector.tensor_tensor(out=zs, in0=zp[:], in1=bout[:],
                                                op=mybir.AluOpType.add)
                    else:
                        nc.scalar.activation(zs, zp[:], AF.Copy)
                    if b == nblocks - 1 or b - zb0 == ZCH - 1:
                        nc.sync.dma_start(
                            out_d[:, zb0 * wcols : (b + 1) * wcols],
                            zbuf[:, 0 : (b + 1 - zb0) * wcols])

                # software pipeline: DMA leads edge-compute by PD ticks; the
                # agg matmuls and node phase trail by 1 and 2 more, so no
                # engine's in-order queue round-trips within a block.
                PD = int(cfg.get("pd", 2))
                L1 = PD + 1
                L2 = PD + 2
                for i in range(nblocks + L2):
                    if i < nblocks:
                        emit_dma(i)
                    if 0 <= i - PD < nblocks:
                        emit_edge(i - PD)
                    if 0 <= i - L1 < nblocks:
                        emit_agg(i - L1)
                    if 0 <= i - L2 < nblocks:
                        emit_node(i - L2)

    nc.compile()
    return nc


def prep_edges(edge_index, n, ncores):
    """Sort self-looped edges by dst; balanced block->core assignment; per-core
    slot layout. Returns dict with Ts, per-core slot src/dst indices, metas,
    block assignment, and sizes."""
    nblocks_g = -(-n // P)  # global 128-node blocks before core padding
    nblocks = -(-nblocks_g // ncores)
    npad = nblocks * ncores * P
    nblocks_g = npad // P
    nper = nblocks * P

    e0 = np.asarray(edge_index[0], dtype=np.int64)
    e1 = np.asarray(edge_index[1], dtype=np.int64)
    loops = np.arange(n, dtype=np.int64)
    src = np.concatenate([e0, loops])
    dst = np.concatenate([e1, loops])
    order = np.argsort(dst, kind="stable")
    srcs = src[order]
    dsts = dst[order]

    bounds = np.searchsorted(dsts, np.arange(0, npad + 1, P))
    cnt = bounds[1:] - bounds[:-1]  # [nblocks_g]

    # Balance: sort blocks by count desc, deal round-robin to cores so each
    # position holds 8 near-equal counts; Ts[b] = ceil(max/128).
    rank = np.argsort(-cnt, kind="stable")
    gmap = rank.reshape(nblocks, ncores)  # gmap[b, c] = global block
    Ts = np.maximum(1, -(-cnt[gmap[:, 0]] // P)).astype(int)
    TOT = int(Ts.sum())
    offs = np.concatenate([[0], np.cumsum(Ts)]).astype(int)

    src_slots, dst_slots, metas = [], [], []
    for c in range(ncores):
        ss = np.zeros(TOT * P, dtype=np.int64)
        ds_ = np.zeros(TOT * P, dtype=np.int64)
        meta = np.full((P, TOT), 300.0, dtype=np.float32)
        for b in range(nblocks):
            g = int(gmap[b, c])
            lo, hi = int(bounds[g]), int(bounds[g + 1])
            cnt_b = hi - lo
            base = int(offs[b]) * P
            j = np.arange(cnt_b)
            slot = base + j  # slot = off*128 + t*128 + p with t=j//128, p=j%128
            ss[slot] = srcs[lo:hi]
            ds_[slot] = dsts[lo:hi]
            loc = (dsts[lo:hi] - g * P).astype(np.float32)
            meta[j % P, int(offs[b]) + j // P] = loc
        src_slots.append(ss)
        dst_slots.append(ds_)
        metas.append(meta.astype(bfnp))
    return {
        "Ts": Ts, "offs": offs, "TOT": TOT, "npad": npad, "nper": nper,
        "nblocks": nblocks, "gmap": gmap, "src_slots": src_slots,
        "dst_slots": dst_slots, "metas": metas,
    }


def expand_rows(nrows_full, prep):
    """Feature-major EA per core: ea[p, (off+?)…] built from slot-major gather.

    ea chunk for block b: [P, 136*T] with col (c*T + t) = payload c of edge
    slot (t*128+p)."""
    Ts, offs = prep["Ts"], prep["offs"]
    TOT, nblocks = prep["TOT"], prep["nblocks"]
    eas = []
    for ss, ds_ in zip(prep["src_slots"], prep["dst_slots"]):
        sm = nrows_full[ss]  # [TOT*128, 136] bf16
        sm[:, D + HEADS : ROW] = nrows_full[ds_, D + HEADS : ROW]
        pm = np.empty((P, TOT * ROW), dtype=bfnp)
        for b in range(nblocks):
            T = int(Ts[b])
            off = int(offs[b])
            blk = sm[off * P : (off + T) * P].reshape(T, P, ROW)
            pm[:, off * ROW : (off + T) * ROW] = (
                blk.transpose(1, 2, 0).reshape(P, ROW * T)
            )
        eas.append(pm)
    return eas


def unpack_out(zpm, wcols, nblocks):
    """[P, nblocks*wcols] partition-major -> [nper, wcols] block-row-major."""
    return (
        zpm.reshape(P, nblocks, wcols).transpose(1, 0, 2).reshape(nblocks * P, wcols)
    )


def amat(att):
    A = np.zeros((D, HEADS), dtype=np.float32)
    att = np.asarray(att, dtype=np.float32)
    for h in range(HEADS):
        A[h * (D // HEADS) : (h + 1) * (D // HEADS), h] = att[h]
    return A


_cache = {}


def run_gat(x, edge_index, W1, att_src1, att_dst1, b1, W2, att_src2, att_dst2, b2,
            Wc, bc, n=None, ncores=NCORES, repeat=1):
    global LAST_INFO
    x = np.asarray(x, dtype=np.float32)
    if n is None:
        n = int(x.shape[0])

    t0 = time.time()
    prep = prep_edges(edge_index, n, ncores)
    nper, nblocks, Ts = prep["nper"], prep["nblocks"], prep["Ts"]
    npad, gmap = prep["npad"], prep["gmap"]
    Tmax = int(max(Ts))
    cfg = dict(CONFIG)
    key = (npad, tuple(Ts), ncores, repeat, tuple(sorted(cfg.items())))
    t1 = time.time()
    if key in _cache:
        ncA, ncB, ncC = _cache[key]
    else:
        ncA = build_node_transform(nper, ROW)
        ncB = build_gat_layer(nper, Ts, ROW, is_last=False, repeat=repeat, cfg=cfg)
        ncC = build_gat_layer(nper, Ts, C, is_last=True, repeat=repeat, cfg=cfg)
        _cache[key] = (ncA, ncB, ncC)
    t2 = time.time()

    W1 = np.asarray(W1, dtype=np.float32)
    W2 = np.asarray(W2, dtype=np.float32)
    Wc = np.asarray(Wc, dtype=np.float32)
    w1cat = np.concatenate([W1, W1 @ amat(att_src1), W1 @ amat(att_dst1)], axis=1)
    w2cat = np.concatenate([W2, W2 @ amat(att_src2), W2 @ amat(att_dst2)], axis=1)
    b1r = np.tile(np.asarray(b1, np.float32)[None, :D], (P, 1)).astype(bfnp)
    b2r = np.tile(np.asarray(b2, np.float32)[None, :D], (P, 1)).astype(bfnp)
    bcr = np.tile(np.asarray(bc, np.float32)[None, :], (P, 1))
    iotat = np.repeat(np.arange(P, dtype=np.float32), Tmax)[None, :].repeat(P, axis=0)
    iotat = np.ascontiguousarray(iotat).astype(bfnp)
    ident = np.eye(P, dtype=np.float32).astype(bfnp)

    xp = np.zeros((npad, D), dtype=np.float32)
    xp[:n] = x

    # Launch A
    mapsA = [
        {"xts": np.ascontiguousarray(xp[c * nper : (c + 1) * nper].T).astype(bfnp),
         "wcat": w1cat.astype(bfnp)}
        for c in range(ncores)
    ]
    resA = run_bass_kernel_spmd(ncA, mapsA, list(range(ncores)))
    nrows_full = np.concatenate(
        [unpack_out(resA.results[c]["npm"].view(bfnp), ROW, nblocks)
         for c in range(ncores)], axis=0)
    t3 = time.time()

    # Expansion 1 + Launch B
    eas = expand_rows(nrows_full, prep)
    mapsB = [
        {"ea": eas[c], "meta": prep["metas"][c], "iotat": iotat,
         "wnext": w2cat.astype(bfnp), "brep": b1r,
         "bout": np.zeros((P, ROW), np.float32), "ident": ident}
        for c in range(ncores)
    ]
    resB = run_bass_kernel_spmd(ncB, mapsB, list(range(ncores)))
    # Un-permute to global node order: core c position b holds global block
    # gmap[b, c], while expansion gathers by global node id.
    zrows_full = np.empty((npad, ROW), dtype=bfnp)
    for c in range(ncores):
        zc = unpack_out(resB.results[c]["zpm"].view(bfnp), ROW, nblocks)
        for b in range(nblocks):
            g = int(gmap[b, c])
            zrows_full[g * P : (g + 1) * P] = zc[b * P : (b + 1) * P]
    t4 = time.time()

    # Expansion 2 + Launch C
    eas2 = expand_rows(zrows_full, prep)
    mapsC = [
        {"ea": eas2[c], "meta": prep["metas"][c], "iotat": iotat,
         "wnext": Wc.astype(bfnp), "brep": b2r, "bout": bcr, "ident": ident}
        for c in range(ncores)
    ]
    resC = run_bass_kernel_spmd(ncC, mapsC, list(range(ncores)))
    t5 = time.time()

    # Un-permute: core c position b holds global block gmap[b, c]
    out = np.zeros((npad, C), dtype=np.float32)
    for c in range(ncores):
        zc = unpack_out(resC.results[c]["zpm"], C, nblocks)  # [nper, 40] f32
        for b in range(nblocks):
            g = int(gmap[b, c])
            out[g * P : (g + 1) * P] = zc[b * P : (b + 1) * P]
    out = out[:n]

    LAST_INFO = {
        "prep_s": t1 - t0, "build_s": t2 - t1, "launchA_s": t3 - t2,
        "launchB_s": t4 - t3, "launchC_s": t5 - t4,
        "ncs": (ncA, ncB, ncC),
        "maps": (mapsA, mapsB, mapsC),
        "nper": nper, "Ts": Ts,
    }
    print(
        f"[kernel] prep={t1 - t0:.2f}s build={t2 - t1:.2f}s A={t3 - t2:.2f}s "
        f"B={t4 - t3:.2f}s C={t5 - t4:.2f}s TOT={prep['TOT']} Tmax={Tmax}",
        file=sys.stderr,
    )
    return out.astype(np.float32)


def kernel(x, edge_index, W1, att_src1, att_dst1, b1, W2, att_src2, att_dst2, b2, Wc, bc):
    return run_gat(x, edge_index, W1, att_src1, att_dst1, b1,
                   W2, att_src2, att_dst2, b2, Wc, bc)


# revision 26
# speedup vs baseline: 1099.8767x; 1.1507x over previous
"""Trainium2 Bass kernel for a 2-layer GAT (4 heads, 32 dim/head) + linear classifier.

Architecture (8 NeuronCores, SPMD; 3 device launches with host-side edge
expansion between them — the host only permutes/replicates rows, all FLOPs
run on device):

- Host prep: append self-loops, sort edges by dst, partition the 392 dst
  node-blocks (128 nodes each) across 8 cores x 49 positions, balancing edge
  counts (blocks sorted by count, dealt round-robin) to minimize the shared
  per-position tile count Ts[b] = ceil(max-core count/128).

- Launch A (node transform): each core computes rows[n] = x[n] @ [W1|W1@As1|W1@Ad1]
  for its 6272-node shard, all bf16 (output rows [h | a_src | a_dst], 136 cols).

- Host expansion (per layer): gather per-edge rows EA[slot] =
  [h[src] | a_src[src] | a_dst[dst]] in a FEATURE-MAJOR layout per block:
  ea[p, (off_b + c)*T_b + t] = payload c of edge slot (b, t, p), bf16.
  Feature-major keeps every device elementwise op's innermost stride == 1,
  which is what unlocks the DVE 2x perf mode (broadcast APs force 1x).

- Launch B/C (GAT layer): per block:
    DMA G [P, 136*T];  ew = a_src+a_dst (DVE 2x);  lrelu+exp on ACT into the
    a_src region (w, head-major);  G.h *= w (DVE 2x, broadcast-free via
    4-dim AP);  one-hot A01[p, d*T+t] = (dstloc==d) via is_equal with a
    materialized wide-iota constant (packed operands -> 2x), split between
    DVE and GPSIMD;  T PE matmuls accumulate U[128,132] = A01^T @ [h*w | w];
    node phase h = relu(U/(s+eps) + b) (DVE + ACT);  PE transpose;
    projection matmul to the next layer's rows (B: bf16 out, C: logits+bc).
  Outputs are partition-major [P, nblocks*wcols] so per-block stores DMA
  contiguous per-partition chunks; the host unpacks.
"""

import os
import sys
import time

for _p in ("/opt/trn_rl_repo", "/root/.axon_site/_ro/trn_rl_repo"):
    if os.path.isdir(_p) and _p not in sys.path:
        sys.path.insert(0, _p)

import dataclasses

import numpy as np
import ml_dtypes

import concourse.bass as bass
import concourse.mybir as mybir
import concourse.tile as tile
from concourse import bacc
from concourse.bass_utils import run_bass_kernel_spmd

P = 128
D = 128
HEADS = 4
C = 40
NEG_SLOPE = 0.2
ROW = D + 2 * HEADS  # 136: [h | a_src | a_dst]
RHS = D + HEADS      # 132: [h*w | w] aggregation rhs width
NCORES = 8

f32 = mybir.dt.float32
bf16 = mybir.dt.bfloat16
bfnp = ml_dtypes.bfloat16

LAST_INFO = {}  # timing info stash for test.py

# gwp: number of h*w feature columns (multiple of 32) multiplied on GPSIMD;
# the rest go on DVE. The one-hot is_equal is DVE-only (GPSIMD lacks the op).
CONFIG = {"gwp": 0, "wbufs": 6, "gbufs": 5, "ubufs": 4}
for _k in ("gwp", "wbufs", "gbufs", "ubufs"):
    if os.environ.get(f"GAT_{_k.upper()}"):
        CONFIG[_k] = int(os.environ[f"GAT_{_k.upper()}"])


def _ap_with(ap, dims):
    return dataclasses.replace(ap, ap=dims)


def build_node_transform(nper, wcols, repeat=1, trn_type="TRN2"):
    """Launch A: rows[n] = xTs[:, n].T @ wcat, partition-major output
    npm[p, i*wcols + c] = rows[i*128 + p, c]; single wide in/out DMAs."""
    nc = bacc.Bacc(trn_type, target_bir_lowering=False, debug=False, num_devices=NCORES)
    xts_d = nc.dram_tensor("xts", [P, nper], bf16, kind="ExternalInput")
    wcat_d = nc.dram_tensor("wcat", [D, wcols], bf16, kind="ExternalInput")
    out_d = nc.dram_tensor("npm", [P, (nper // P) * wcols], bf16,
                           kind="ExternalOutput")
    nt = nper // P
    with tile.TileContext(nc) as tc:
        with (
            tc.tile_pool(name="const", bufs=1) as cpool,
            tc.tile_pool(name="work", bufs=2) as pool,
            tc.tile_pool(name="psum", bufs=4, space="PSUM") as psum,
        ):
            wcat = cpool.tile([D, wcols], bf16, tag="wcat")
            nc.sync.dma_start(wcat[:], wcat_d[:])
            for rep in range(repeat):
                if rep:
                    tc.strict_bb_all_engine_barrier()
                xts = pool.tile([P, nper], bf16, tag="xts")
                nc.sync.dma_start(xts[:], xts_d[:])
                ob = pool.tile([P, nt * wcols], bf16, tag="ob")
                for i in range(nt):
                    hp = psum.tile([P, wcols], f32, tag="hp")
                    nc.tensor.matmul(hp[:], lhsT=xts[:, i * P : (i + 1) * P],
                                     rhs=wcat[:], start=True, stop=True)
                    nc.vector.tensor_copy(ob[:, i * wcols : (i + 1) * wcols], hp[:])
                nc.sync.dma_start(out_d[:], ob[:])
    nc.compile()
    return nc


def build_gat_layer(nper, Ts, wcols, is_last, repeat=1, trn_type="TRN2", cfg=None):
    """Launch B/C: edge aggregation + node phase for one GAT layer (v2).

    Inputs (all per core): ea [P, TOT*136] bf16 feature-major edge rows,
    meta [P, TOT] bf16 dst-local ids (pad 300), iotat [P, 128*Tmax] bf16,
    wnext [128, wcols] bf16, brep [P, 128] bf16, bout [P, wcols] f32,
    ident [P, 128] bf16.
    Output: zpm [P, nblocks*wcols] partition-major (bf16 for B, f32 for C).
    """
    cfg = dict(CONFIG if cfg is None else cfg)
    gwp = int(cfg["gwp"])
    assert gwp % (D // HEADS) == 0
    abl = set(cfg.get("ablate", ()))  # timing-only ablations, break numerics
    ZCH = 8  # blocks per output-DMA chunk
    nblocks = nper // P
    assert len(Ts) == nblocks
    Ts = [int(t) for t in Ts]
    TOT = int(sum(Ts))
    Tmax = int(max(Ts))
    offs = np.concatenate([[0], np.cumsum(Ts)]).astype(int)
    odt = f32 if is_last else bf16

    nc = bacc.Bacc(trn_type, target_bir_lowering=False, debug=False, num_devices=NCORES)
    ea_d = nc.dram_tensor("ea", [P, TOT * ROW], bf16, kind="ExternalInput")
    meta_d = nc.dram_tensor("meta", [P, TOT], bf16, kind="ExternalInput")
    iotat_d = nc.dram_tensor("iotat", [P, P * Tmax], bf16, kind="ExternalInput")
    wnext_d = nc.dram_tensor("wnext", [D, wcols], bf16, kind="ExternalInput")
    brep_d = nc.dram_tensor("brep", [P, D], bf16, kind="ExternalInput")
    bout_d = nc.dram_tensor("bout", [P, wcols], f32, kind="ExternalInput")
    ident_d = nc.dram_tensor("ident", [P, P], bf16, kind="ExternalInput")
    out_d = nc.dram_tensor("zpm", [P, nblocks * wcols], odt, kind="ExternalOutput")

    AF = mybir.ActivationFunctionType

    with tile.TileContext(nc) as tc:
        with (
            tc.tile_pool(name="const", bufs=1) as cpool,
            tc.tile_pool(name="work", bufs=int(cfg["wbufs"])) as pool,
            tc.tile_pool(name="gath", bufs=int(cfg["gbufs"])) as gpool,
            tc.tile_pool(name="psum", bufs=2, space="PSUM") as psum,
            tc.tile_pool(name="psU", bufs=int(cfg["ubufs"]), space="PSUM") as psU,
        ):
            wnext = cpool.tile([D, wcols], bf16, tag="wnext")
            nc.sync.dma_start(wnext[:], wnext_d[:])
            brep = cpool.tile([P, D], bf16, tag="brep")
            nc.sync.dma_start(brep[:], brep_d[:])
            bout = cpool.tile([P, wcols], f32, tag="bout")
            nc.sync.dma_start(bout[:], bout_d[:])
            ident = cpool.tile([P, P], bf16, tag="ident")
            nc.sync.dma_start(ident[:], ident_d[:])
            iotat = cpool.tile([P, P * Tmax], bf16, tag="iotat")
            nc.sync.dma_start(iotat[:], iotat_d[:])
            meta = cpool.tile([P, TOT], bf16, tag="meta")
            nc.sync.dma_start(meta[:], meta_d[:])

            sp_iota = iotat[:].ap[0]
            H32 = D // HEADS

            for rep in range(repeat):
                if rep:
                    tc.strict_bb_all_engine_barrier()
                sts = {}
                zstate = {}

                def emit_dma(b):
                    T = Ts[b]
                    off = int(offs[b])
                    G = gpool.tile([P, ROW * T], bf16, tag="G")
                    if "dma" not in abl or (b == 0 and rep == 0):
                        nc.sync.dma_start(G[:], ea_d[:, off * ROW : (off + T) * ROW])
                    sts[b] = {"G": G, "T": T, "off": off}

                def emit_edge(b):
                    st = sts[b]
                    G, T, off = st["G"], st["T"], st["off"]
                    acol = G[:, D * T : (D + HEADS) * T]       # a_src, [P, 4T]
                    dcol = G[:, (D + HEADS) * T : ROW * T]     # a_dst, [P, 4T]
                    # e = a_src + a_dst; w = exp(leakyrelu(e)) -> over a_src
                    if "act" not in abl:
                        ew = pool.tile([P, HEADS * T], bf16, tag="ew")
                        nc.vector.tensor_tensor(out=ew[:], in0=acol, in1=dcol,
                                                op=mybir.AluOpType.add)
                        # leaky-relu via mult+max (ACT Lrelu ignores alpha)
                        lk = pool.tile([P, HEADS * T], bf16, tag="lk")
                        nc.vector.tensor_scalar_mul(lk[:], ew[:], NEG_SLOPE)
                        nc.vector.tensor_tensor(out=ew[:], in0=ew[:], in1=lk[:],
                                                op=mybir.AluOpType.max)
                        nc.scalar.activation(acol, ew[:], AF.Exp)
                    # h *= w (head-broadcast AP; packed last dims -> DVE 2x)
                    if "gw" not in abl:
                        for eng, c0, c1 in ((nc.vector, 0, D - gwp),
                                            (nc.gpsimd, D - gwp, D)):
                            if c1 <= c0:
                                continue
                            hpart = G[:, c0 * T : c1 * T]
                            wpart = _ap_with(G[:, (D + c0 // H32) * T :],
                                             [G[:].ap[0], [T, (c1 - c0) // H32],
                                              [0, H32], [1, T]])
                            eng.tensor_tensor(out=hpart, in0=hpart, in1=wpart,
                                              op=mybir.AluOpType.mult)
                    # one-hot A01[p, d*T + t] = (dstloc[p,t] == d)  (DVE 2x)
                    A01 = gpool.tile([P, P * T], bf16, tag="A01")
                    if "a01" in abl:
                        nc.vector.memset(A01[:], 0.0)
                    else:
                        m_b = meta[:, off : off + T]
                        i0 = _ap_with(m_b, [m_b.ap[0], [0, P], [1, T]])
                        i1 = _ap_with(iotat[:], [sp_iota, [Tmax, P], [1, T]])
                        o = _ap_with(A01[:], [A01[:].ap[0], [T, P], [1, T]])
                        nc.vector.tensor_tensor(out=o, in0=i0, in1=i1,
                                                op=mybir.AluOpType.is_equal)
                    st["A01"] = A01

                def emit_agg(b):
                    st = sts[b]
                    G, A01, T = st["G"], st["A01"], st["T"]
                    U = psU.tile([P, RHS], f32, tag="U")
                    for t in range(1 if "mm" in abl else T):
                        lhsT_t = _ap_with(A01[:, t:], [A01[:].ap[0], [T, P]])
                        rhs_t = _ap_with(G[:, t:], [G[:].ap[0], [T, RHS]])
                        nc.tensor.matmul(U[:], lhsT=lhsT_t, rhs=rhs_t,
                                         start=(t == 0),
                                         stop=(t == T - 1) or "mm" in abl)
                    st["U"] = U

                def emit_node(b):
                    st = sts.pop(b)
                    U = st["U"]
                    # node phase: h = relu(U/s + bias); s>0 for any node with
                    # an edge (self-loops guarantee it); pad rows give nan but
                    # are never gathered downstream.
                    hf = pool.tile([P, D], bf16, tag="hf")
                    if "node" in abl:
                        nc.vector.memset(hf[:], 0.0)
                    else:
                        rcp = pool.tile([P, HEADS], f32, tag="rcp")
                        nc.vector.reciprocal(rcp[:], U[:, D : D + HEADS])
                        h = pool.tile([P, D], bf16, tag="h")
                        rexp = _ap_with(rcp[:],
                                        [rcp[:].ap[0], [1, HEADS], [0, H32]])
                        nc.vector.tensor_tensor(out=h[:], in0=U[:, 0:D], in1=rexp,
                                                op=mybir.AluOpType.mult)
                        nc.vector.tensor_tensor(out=h[:], in0=h[:], in1=brep[:],
                                                op=mybir.AluOpType.add)
                        nc.scalar.activation(hf[:], h[:], AF.Relu)
                    # project: z = hf @ wnext (+ bout for the classifier)
                    hTp = psum.tile([P, P], bf16, tag="hTp")
                    nc.tensor.transpose(hTp[:], hf[:], ident[:])
                    hT = pool.tile([P, P], bf16, tag="hT")
                    nc.scalar.activation(hT[:], hTp[:], AF.Copy)
                    zp = psum.tile([P, wcols], f32, tag="zp")
                    nc.tensor.matmul(zp[:], lhsT=hT[:], rhs=wnext[:], start=True,
                                     stop=True)
                    if b % ZCH == 0:
                        zbuf = pool.tile([P, ZCH * wcols], odt, tag="zbuf")
                        zstate["b0"] = b
                        zstate["buf"] = zbuf
                    zb0, zbuf = zstate["b0"], zstate["buf"]
                    zs = zbuf[:, (b - zb0) * wcols : (b - zb0 + 1) * wcols]
                    if is_last:
                        nc.vector.tensor_tensor(out=zs, in0=zp[:], in1=bout[:],
                                                op=mybir.AluOpType.add)
                    else:
                        nc.scalar.activation(zs, zp[:], AF.Copy)
                    if b == nblocks - 1 or b - zb0 == ZCH - 1:
                        nc.sync.dma_start(
                            out_d[:, zb0 * wcols : (b + 1) * wcols],
                            zbuf[:, 0 : (b + 1 - zb0) * wcols])

                # software pipeline: DMA leads edge-compute by PD ticks; the
                # agg matmuls and node phase trail by 1 and 2 more, so no
                # engine's in-order queue round-trips within a block.
                PD = int(cfg.get("pd", 2))
                L1 = PD + 1
                L2 = PD + 2
                for i in range(nblocks + L2):
                    if i < nblocks:
                        emit_dma(i)
                    if 0 <= i - PD < nblocks:
                        emit_edge(i - PD)
                    if 0 <= i - L1 < nblocks:
                        emit_agg(i - L1)
                    if 0 <= i - L2 < nblocks:
                        emit_node(i - L2)

    nc.compile()
    return nc


def prep_edges(edge_index, n, ncores):
    """Sort self-looped edges by dst; balanced block->core assignment; per-core
    slot layout. Returns dict with Ts, per-core slot src/dst indices, metas,
    block assignment, and sizes."""
    nblocks_g = -(-n // P)  # global 128-node blocks before core padding
    nblocks = -(-nblocks_g // ncores)
    npad = nblocks * ncores * P
    nblocks_g = npad // P
    nper = nblocks * P

    e0 = np.asarray(edge_index[0], dtype=np.int64)
    e1 = np.asarray(edge_index[1], dtype=np.int64)
    loops = np.arange(n, dtype=np.int64)
    src = np.concatenate([e0, loops])
    dst = np.concatenate([e1, loops])
    order = np.argsort(dst, kind="stable")
    srcs = src[order]
    dsts = dst[order]

    bounds = np.searchsorted(dsts, np.arange(0, npad + 1, P))
    cnt = bounds[1:] - bounds[:-1]  # [nblocks_g]

    # Balance: sort blocks by count desc, deal round-robin to cores so each
    # position holds 8 near-equal counts; Ts[b] = ceil(max/128).
    rank = np.argsort(-cnt, kind="stable")
    gmap = rank.reshape(nblocks, ncores)  # gmap[b, c] = global block
    Ts = np.maximum(1, -(-cnt[gmap[:, 0]] // P)).astype(int)
    TOT = int(Ts.sum())
    offs = np.concatenate([[0], np.cumsum(Ts)]).astype(int)

    src_slots, dst_slots, metas = [], [], []
    for c in range(ncores):
        ss = np.zeros(TOT * P, dtype=np.int64)
        ds_ = np.zeros(TOT * P, dtype=np.int64)
        meta = np.full((P, TOT), 300.0, dtype=np.float32)
        for b in range(nblocks):
            g = int(gmap[b, c])
            lo, hi = int(bounds[g]), int(bounds[g + 1])
            cnt_b = hi - lo
            base = int(offs[b]) * P
            j = np.arange(cnt_b)
            slot = base + j  # slot = off*128 + t*128 + p with t=j//128, p=j%128
            ss[slot] = srcs[lo:hi]
            ds_[slot] = dsts[lo:hi]
            loc = (dsts[lo:hi] - g * P).astype(np.float32)
            meta[j % P, int(offs[b]) + j // P] = loc
        src_slots.append(ss)
        dst_slots.append(ds_)
        metas.append(meta.astype(bfnp))
    return {
        "Ts": Ts, "offs": offs, "TOT": TOT, "npad": npad, "nper": nper,
        "nblocks": nblocks, "gmap": gmap, "src_slots": src_slots,
        "dst_slots": dst_slots, "metas": metas,
    }


def expand_rows(nrows_full, prep):
    """Feature-major EA per core: ea[p, (off+?)…] built from slot-major gather.

    ea chunk for block b: [P, 136*T] with col (c*T + t) = payload c of edge
    slot (t*128+p)."""
    Ts, offs = prep["Ts"], prep["offs"]
    TOT, nblocks = prep["TOT"], prep["nblocks"]
    eas = []
    for ss, ds_ in zip(prep["src_slots"], prep["dst_slots"]):
        sm = nrows_full[ss]  # [TOT*128, 136] bf16
        sm[:, D + HEADS : ROW] = nrows_full[ds_, D + HEADS : ROW]
        pm = np.empty((P, TOT * ROW), dtype=bfnp)
        for b in range(nblocks):
            T = int(Ts[b])
            off = int(offs[b])
            blk = sm[off * P : (off + T) * P].reshape(T, P, ROW)
            pm[:, off * ROW : (off + T) * ROW] = (
                blk.transpose(1, 2, 0).reshape(P, ROW * T)
            )
        eas.append(pm)
    return eas


def unpack_out(zpm, wcols, nblocks):
    """[P, nblocks*wcols] partition-major -> [nper, wcols] block-row-major."""
    return (
        zpm.reshape(P, nblocks, wcols).transpose(1, 0, 2).reshape(nblocks * P, wcols)
    )


def amat(att):
    A = np.zeros((D, HEADS), dtype=np.float32)
    att = np.asarray(att, dtype=np.float32)
    for h in range(HEADS):
        A[h * (D // HEADS) : (h + 1) * (D // HEADS), h] = att[h]
    return A


_cache = {}


def run_gat(x, edge_index, W1, att_src1, att_dst1, b1, W2, att_src2, att_dst2, b2,
            Wc, bc, n=None, ncores=NCORES, repeat=1):
    global LAST_INFO
    x = np.asarray(x, dtype=np.float32)
    if n is None:
        n = int(x.shape[0])

    t0 = time.time()
    prep = prep_edges(edge_index, n, ncores)
    nper, nblocks, Ts = prep["nper"], prep["nblocks"], prep["Ts"]
    npad, gmap = prep["npad"], prep["gmap"]
    Tmax = int(max(Ts))
    cfg = dict(CONFIG)
    key = (npad, tuple(Ts), ncores, repeat, tuple(sorted(cfg.items())))
    t1 = time.time()
    if key in _cache:
        ncA, ncB, ncC = _cache[key]
    else:
        ncA = build_node_transform(nper, ROW)
        ncB = build_gat_layer(nper, Ts, ROW, is_last=False, repeat=repeat, cfg=cfg)
        ncC = build_gat_layer(nper, Ts, C, is_last=True, repeat=repeat, cfg=cfg)
        _cache[key] = (ncA, ncB, ncC)
    t2 = time.time()

    W1 = np.asarray(W1, dtype=np.float32)
    W2 = np.asarray(W2, dtype=np.float32)
    Wc = np.asarray(Wc, dtype=np.float32)
    w1cat = np.concatenate([W1, W1 @ amat(att_src1), W1 @ amat(att_dst1)], axis=1)
    w2cat = np.concatenate([W2, W2 @ amat(att_src2), W2 @ amat(att_dst2)], axis=1)
    b1r = np.tile(np.asarray(b1, np.float32)[None, :D], (P, 1)).astype(bfnp)
    b2r = np.tile(np.asarray(b2, np.float32)[None, :D], (P, 1)).astype(bfnp)
    bcr = np.tile(np.asarray(bc, np.float32)[None, :], (P, 1))
    iotat = np.repeat(np.arange(P, dtype=np.float32), Tmax)[None, :].repeat(P, axis=0)
    iotat = np.ascontiguousarray(iotat).astype(bfnp)
    ident = np.eye(P, dtype=np.float32).astype(bfnp)

    xp = np.zeros((npad, D), dtype=np.float32)
    xp[:n] = x

    # Launch A
    mapsA = [
        {"xts": np.ascontiguousarray(xp[c * nper : (c + 1) * nper].T).astype(bfnp),
         "wcat": w1cat.astype(bfnp)}
        for c in range(ncores)
    ]
    resA = run_bass_kernel_spmd(ncA, mapsA, list(range(ncores)))
    nrows_full = np.concatenate(
        [unpack_out(resA.results[c]["npm"].view(bfnp), ROW, nblocks)
         for c in range(ncores)], axis=0)
    t3 = time.time()

    # Expansion 1 + Launch B
    eas = expand_rows(nrows_full, prep)
    mapsB = [
        {"ea": eas[c], "meta": prep["metas"][c], "iotat": iotat,
         "wnext": w2cat.astype(bfnp), "brep": b1r,
         "bout": np.zeros((P, ROW), np.float32), "ident": ident}
        for c in range(ncores)
    ]
    resB = run_bass_kernel_spmd(ncB, mapsB, list(range(ncores)))
    # Un-permute to global node order: core c position b holds global block
    # gmap[b, c], while expansion gathers by global node id.
    zrows_full = np.empty((npad, ROW), dtype=bfnp)
    for c in range(ncores):
        zc = unpack_out(resB.results[c]["zpm"].view(bfnp), ROW, nblocks)
        for b in range(nblocks):
            g = int(gmap[b, c])
            zrows_full[g * P : (g + 1) * P] = zc[b * P : (b + 1) * P]
    t4 = time.time()

    # Expansion 2 + Launch C
    eas2 = expand_rows(zrows_full, prep)
    mapsC = [
        {"ea": eas2[c], "meta": prep["metas"][c], "iotat": iotat,
         "wnext": Wc.astype(bfnp), "brep": b2r, "bout": bcr, "ident": ident}
        for c in range(ncores)
    ]
    resC = run_bass_kernel_spmd(ncC, mapsC, list(range(ncores)))
    t5 = time.time()

    # Un-permute: core c position b holds global block gmap[b, c]
    out = np.zeros((npad, C), dtype=np.float32)
    for c in range(ncores):
        zc = unpack_out(resC.results[c]["zpm"], C, nblocks)  # [nper, 40] f32
        for b in range(nblocks):
            g = int(gmap[b, c])
            out[g * P : (g + 1) * P] = zc[b * P : (b + 1) * P]
    out = out[:n]

    LAST_INFO = {
        "prep_s": t1 - t0, "build_s": t2 - t1, "launchA_s": t3 - t2,
        "launchB_s": t4 - t3, "launchC_s": t5 - t4,
        "ncs": (ncA, ncB, ncC),
        "maps": (mapsA, mapsB, mapsC),
        "nper": nper, "Ts": Ts,
    }
    print(
        f"[kernel] prep={t1 - t0:.2f}s build={t2 - t1:.2f}s A={t3 - t2:.2f}s "
        f"B={t4 - t3:.2f}s C={t5 - t4:.2f}s TOT={prep['TOT']} Tmax={Tmax}",
        file=sys.stderr,
    )
    return out.astype(np.float32)


def kernel(x, edge_index, W1, att_src1, att_dst1, b1, W2, att_src2, att_dst2, b2, Wc, bc):
    return run_gat(x, edge_index, W1, att_src1, att_dst1, b1,
                   W2, att_src2, att_dst2, b2, Wc, bc)
